# revision 25
# baseline (speedup 1.0000x reference)
import sys

sys.path.insert(0, "/opt/trn_rl_repo")

import numpy as np
from contextlib import ExitStack

import concourse.bass as bass
import concourse.bacc as bacc
import concourse.tile as tile
from concourse import mybir
from concourse.bass_utils import run_bass_kernel_spmd
from concourse.masks import make_identity

B, C, H, W = 16, 64, 64, 64
HW = H * W          # 4096
M = HW // 4         # 1024
NCORES = 8
BPC = B // NCORES   # batches per core
F32 = mybir.dt.float32
BF16 = mybir.dt.bfloat16
FP8 = mybir.dt.float8e4

NCHUNK = 1024       # n-dim chunk (columns of s^T / o)
NCH = HW // NCHUNK  # 4 chunks per batch
MT = M // 128       # 8 m-tiles of 128


def _build_nc():
    nc = bacc.Bacc(None, target_bir_lowering=False)

    x_d = nc.dram_tensor("x", [BPC, C, HW], F32, kind="ExternalInput")
    xb_d = nc.dram_tensor("xb", [BPC, C, HW], BF16, kind="ExternalInput")
    # [w_phi(8); zeros(24); w_g(32)]^T  -> proj rows: 0:8 g, 32:64 h
    wpgt_d = nc.dram_tensor("wpgt", [C, C], BF16, kind="ExternalInput")
    wtheta_d = nc.dram_tensor("wtheta", [8, C], BF16, kind="ExternalInput")
    wot_d = nc.dram_tensor("wot", [32, C], BF16, kind="ExternalInput")
    out_d = nc.dram_tensor("out", [BPC, C, HW], F32, kind="ExternalOutput")
    rd_d = nc.dram_tensor("rd", [BPC, HW], F32)       # recip bounce (internal)

    with tile.TileContext(nc) as tc, ExitStack() as ctx:
        consts = ctx.enter_context(tc.tile_pool(name="consts", bufs=1))
        wpgt_sb = consts.tile([C, C], BF16)
        wtheta_sb = consts.tile([8, C], BF16)
        wot_sb = consts.tile([C, C], BF16)   # rows 32:64 hold (gamma*w_o)^T
        ident65 = consts.tile([65, 65], BF16)
        ones_sb = consts.tile([1, NCHUNK], BF16)

        # SBUF pools
        xp = ctx.enter_context(tc.tile_pool(name="xp", bufs=2))
        xbp = ctx.enter_context(tc.tile_pool(name="xbp", bufs=2))
        poolp = ctx.enter_context(tc.tile_pool(name="poolp", bufs=2))
        hwp = ctx.enter_context(tc.tile_pool(name="hwp", bufs=2))
        Gp = ctx.enter_context(tc.tile_pool(name="Gp", bufs=2))
        hTp = ctx.enter_context(tc.tile_pool(name="hTp", bufs=2))
        expp = ctx.enter_context(tc.tile_pool(name="expp", bufs=6))
        rbp = ctx.enter_context(tc.tile_pool(name="rbp", bufs=2))
        smallp = ctx.enter_context(tc.tile_pool(name="smallp", bufs=4))
        tmulp = ctx.enter_context(tc.tile_pool(name="tmulp", bufs=2))
        o65p = ctx.enter_context(tc.tile_pool(name="o65p", bufs=2))
        outp = ctx.enter_context(tc.tile_pool(name="outp", bufs=2))

        # PSUM pools (8 banks):
        #   psSA: 2 x [128,1024]f32 = 4 banks (proj pp + sT ring)
        #   psA : 2 x 1 bank        = 2 banks (G/hw/ht half-tiles ring)
        #   psO : 1 x [65,1024]f32  = 2 banks (o accumulation)
        psSA = ctx.enter_context(tc.tile_pool(name="psSA", bufs=2, space="PSUM"))
        psA = ctx.enter_context(tc.tile_pool(name="psA", bufs=2, space="PSUM"))
        psO = ctx.enter_context(tc.tile_pool(name="psO", bufs=1, space="PSUM"))

        # ---- input DMAs spread across queues/rings ----
        # sync ring: xb(b0) chunk 0 first, weights, then rest of xb(b0)
        xb0 = xbp.tile([128, HW], BF16, name="xb0")
        nc.sync.dma_start(out=xb0[0:C, 0:NCHUNK], in_=xb_d[0][:, 0:NCHUNK])
        nc.sync.dma_start(out=xb0[C:128, 0:NCHUNK], in_=xb_d[0][:, 0:NCHUNK])
        nc.sync.dma_start(out=wpgt_sb, in_=wpgt_d[:])
        nc.sync.dma_start(out=wtheta_sb, in_=wtheta_d[:])
        nc.sync.dma_start(out=wot_sb[32:64, :], in_=wot_d[:])
        make_identity(nc, ident65)
        nc.vector.memset(ones_sb, 1.0)
        for k in range(1, NCH):
            cs = slice(k * NCHUNK, (k + 1) * NCHUNK)
            nc.sync.dma_start(out=xb0[0:C, cs], in_=xb_d[0][:, cs])
            nc.sync.dma_start(out=xb0[C:128, cs], in_=xb_d[0][:, cs])
        # scalar ring: xb(b1); tensor ring: x fp32
        xb1 = xbp.tile([128, HW], BF16, name="xb1")
        nc.scalar.dma_start(out=xb1[0:C, :], in_=xb_d[1])
        nc.scalar.dma_start(out=xb1[C:128, :], in_=xb_d[1])
        xb_sbs = [xb0, xb1]
        x_sbs = []
        for b in range(BPC):
            x_sb = xp.tile([C, HW], F32, name=f"x{b}")
            nc.scalar.dma_start(out=x_sb, in_=x_d[b])
            x_sbs.append(x_sb)

        pend = [None]  # deferred (o65, ck, b, recipB) output emission

        def emit_out():
            o65, pck, pb, recipB = pend[0]
            pend[0] = None
            t = tmulp.tile([C, NCHUNK], F32, name="tmul")
            nc.vector.tensor_mul(t, o65[0:C, :], recipB)
            outc = outp.tile([C, NCHUNK], F32, name="outc")
            nc.vector.tensor_add(outc, t, x_sbs[pb][:, pck])
            nc.gpsimd.dma_start(out=out_d[pb, :, pck], in_=outc)

        # ---- phase A pieces (per batch, per pooling chunk k) ----
        state = {}

        def phaseA_piece(b, k):
            xb_sb = xb_sbs[b]
            if k == 0:
                st = state[b] = {}
                st["pooled"] = poolp.tile([C, H // 2, W // 2], BF16,
                                          name=f"pool{b}")
                st["G"] = Gp.tile([128, M], BF16, name=f"G{b}")
                st["hw65"] = hwp.tile([65, M], BF16, name=f"hw65_{b}")
                st["hT8"] = hTp.tile([128, MT // 2, 2, 80], FP8,
                                     name=f"hT8_{b}")
                nc.gpsimd.dma_start(out=st["hw65"][C:C + 1, :], in_=ones_sb)
            st = state[b]
            pooled, G_sb, hw65, hT8 = (st["pooled"], st["G"], st["hw65"],
                                       st["hT8"])
            pp = psSA.tile([C, NCHUNK], F32, name="pp", tag="sa")
            for j in range(2):
                sl = slice(k * NCHUNK + j * 512, k * NCHUNK + (j + 1) * 512)
                nc.tensor.matmul(pp[:, j * 512:(j + 1) * 512], wpgt_sb,
                                 xb_sb[0:C, sl], start=True, stop=True)
            pp5 = pp.rearrange("c (h2 th w2 tw) -> c h2 w2 th tw",
                               h2=8, th=2, w2=W // 2, tw=2)
            nc.vector.tensor_reduce(
                pooled[:, 8 * k:8 * (k + 1), :], pp5,
                axis=mybir.AxisListType.XY, op=mybir.AluOpType.max,
            )
            return

        def phaseA_trio(b, hf):
            st = state[b]
            pooled, G_sb, hw65, hT8 = (st["pooled"], st["G"], st["hw65"],
                                       st["hT8"])
            # half hf covers pooled cols [hf*512, hf*512+512)
            msl = slice(hf * 512, (hf + 1) * 512)
            # G half: duplicated into [128, 512] via column tiles
            Gph = psA.tile([128, 512], F32, name="Gph", tag="a")
            gflat = pooled[0:8].rearrange("c h w -> c (h w)")
            nc.tensor.matmul(Gph[0:C, :], wtheta_sb, gflat[:, msl],
                             start=True, stop=True, tile_position=(0, 0))
            nc.tensor.matmul(Gph[C:128, :], wtheta_sb, gflat[:, msl],
                             start=True, stop=True, tile_position=(0, 64))
            nc.vector.tensor_copy(G_sb[:, msl], Gph)
            # hw half = (gamma*w_o) @ h_pooled (PE rows 32:64)
            hwph = psA.tile([C, 512], F32, name="hwph", tag="a")
            hflat = pooled[32:64].rearrange("c h w -> c (h w)")
            nc.tensor.matmul(hwph, wot_sb[32:64, :], hflat[:, msl],
                             start=True, stop=True)
            nc.vector.tensor_copy(hw65[0:C, msl], hwph)
            # transposes for m-tiles of this half -> fp8 DoubleRow layout
            htph = psA.tile([128, 2, 2, 80], BF16, name="htph", tag="a")
            for mtl in range(4):
                mt = hf * 4 + mtl
                mt2l, j = divmod(mtl, 2)
                nc.tensor.transpose(
                    htph[:, mt2l, j, 0:65],
                    hw65[:, mt * 128:(mt + 1) * 128],
                    ident65,
                )
            nc.vector.tensor_copy(
                hT8[:, hf * 2:hf * 2 + 2, :, 0:65], htph[:, :, :, 0:65])

        # ---- phase B chunk ----
        def chunk(b, k):
            xb_sb = xb_sbs[b]
            st = state[b]
            G_sb, hT8 = st["G"], st["hT8"]
            ck = slice(k * NCHUNK, (k + 1) * NCHUNK)
            o_ps = psO.tile([65, NCHUNK], F32, name="o_ps", tag="o")
            expTs = []
            for mt2 in range(MT // 2):
                expT = expp.tile([128, 2, NCHUNK], FP8,
                                 name=f"expT{mt2}", tag="exp")
                expTs.append(expT)
                sTs = [psSA.tile([128, NCHUNK], F32, name=f"sT{j}", tag="sa")
                       for j in range(2)]
                # row-tiled pair: j=0 on PE rows 0-63, j=1 on rows 64-127
                for jj in range(2):
                    sl = slice(k * NCHUNK + jj * 512,
                               k * NCHUNK + (jj + 1) * 512)
                    osl = slice(jj * 512, (jj + 1) * 512)
                    for j in range(2):
                        mt = 2 * mt2 + j
                        pb_ = j * C
                        nc.tensor.matmul(
                            sTs[j][:, osl],
                            G_sb[pb_:pb_ + C, mt * 128:(mt + 1) * 128],
                            xb_sb[pb_:pb_ + C, sl],
                            start=True, stop=True,
                        )
                for j in range(2):
                    nc.scalar.activation(
                        expT[:, j, :], sTs[j],
                        func=mybir.ActivationFunctionType.Exp,
                    )
                if mt2 == 1 and pend[0] is not None:
                    emit_out()
            # D matmuls after all sT pairs (PE queue is in-order; these
            # wait on exps, so they must not block later sT issue)
            for mt2 in range(MT // 2):
                for jj in range(2):
                    nc.tensor.matmul(
                        o_ps[:, jj * 512:(jj + 1) * 512],
                        hT8[:, mt2, :, 0:65],
                        expTs[mt2][:, :, jj * 512:(jj + 1) * 512],
                        start=(mt2 == 0), stop=(mt2 == MT // 2 - 1),
                        perf_mode=mybir.MatmulPerfMode.DoubleRow,
                    )
            # evacuate o (DMA cannot read PSUM); frees the PSUM bank
            o65 = o65p.tile([65, NCHUNK], F32, name="o65")
            nc.vector.tensor_copy(o65, o_ps)
            # sumexp row -> [128, 8] (direct SBUF->SBUF repartition)
            rs = smallp.tile([128, NCHUNK // 128], F32, name="rs")
            nc.gpsimd.dma_start(
                out=rs, in_=o65[C:C + 1, :].rearrange("o (p i) -> o p i",
                                                      p=128))
            rr = smallp.tile([128, NCHUNK // 128], F32, name="rr")
            nc.vector.reciprocal(rr, rs)
            nc.gpsimd.dma_start(
                out=rd_d[b, ck].rearrange("(p i) -> p i", p=128), in_=rr)
            recipB = rbp.tile([C, NCHUNK], F32, name="recipB")
            rd_ck = rd_d[b, ck]
            nc.gpsimd.dma_start(
                out=recipB,
                in_=bass.AP(tensor=rd_ck.tensor, offset=rd_ck.offset,
                            ap=[[0, C]] + list(rd_ck.ap)),
            )
            pend[0] = (o65, ck, b, recipB)

        # ---- schedule: phase A(b0) pipelined, then b0 chunks with
        # phase A(b1) pieces interleaved, then b1 chunks ----
        for k in range(NCH):
            phaseA_piece(0, k)
            if k >= 2:
                phaseA_trio(0, k - 2)
        for k in range(NCH):
            chunk(0, k)
            phaseA_piece(1, k)
            if k % 2 == 1:
                phaseA_trio(1, k // 2)
        for k in range(NCH):
            chunk(1, k)
        emit_out()

    if not nc.is_finalized():
        nc.finalize()
    return nc


_NC_CACHE = {}


def _run(inputs: dict, trace: bool = False):
    if "nc" not in _NC_CACHE:
        _NC_CACHE["nc"] = _build_nc()
    nc = _NC_CACHE["nc"]

    import ml_dtypes

    x = np.ascontiguousarray(inputs["x"], dtype=np.float32).reshape(B, C, HW)
    xb16 = x.astype(ml_dtypes.bfloat16)
    wpgt = np.ascontiguousarray(
        np.concatenate(
            [inputs["w_phi"], np.zeros((24, C), np.float32), inputs["w_g"]],
            axis=0,
        ).T.astype(ml_dtypes.bfloat16)
    )
    wtheta = np.ascontiguousarray(
        np.asarray(inputs["w_theta"]).astype(ml_dtypes.bfloat16)
    )
    wot = np.ascontiguousarray(
        (float(inputs["gamma"][0]) * inputs["w_o"]).T.astype(ml_dtypes.bfloat16)
    )

    in_maps = []
    for i in range(NCORES):
        in_maps.append({
            "x": np.ascontiguousarray(x[i * BPC:(i + 1) * BPC]),
            "xb": np.ascontiguousarray(xb16[i * BPC:(i + 1) * BPC]),
            "wpgt": wpgt,
            "wtheta": wtheta,
            "wot": wot,
        })

    res = run_bass_kernel_spmd(nc, in_maps, list(range(NCORES)), trace=trace)
    out = np.concatenate([r["out"] for r in res.results], axis=0)
    return out.reshape(B, C, H, W).astype(np.float32), res


def kernel(**inputs):
    out, _ = _run(inputs, trace=False)
    return out
